# revision 9
# baseline (speedup 1.0000x reference)
"""Trainium2 Bass kernel: bilinear grid_sample (align_corners=True).

reference: coord [N,2] in [-1,1], params [1,32,1024,1024] -> out [N,32].

Strategy (8 NeuronCores, data-parallel over queries):
  - Host: build a bf16 "pair table" [H*W + pad, 64]: row (y*1024 + x) holds
    the 32 channels of grid pixel (y, x) followed by the 32 channels of
    (y+1, x) (clamped at the bottom edge). Two consecutive rows therefore
    hold all FOUR bilinear corners of a query whose top-left cell is (y, x)
    as 128 contiguous bf16 = 256 B -> ONE indirect-DMA gather per query
    (the baseline needed two), at half the HBM bytes (bf16 vs fp32).
  - Each core: 250k queries, tiled 128 partitions x kq. DVE computes cell
    indices + the 4 corner weights (bf16); one SWDGE indirect DMA per 128
    queries gathers the corner block; DVE does the weighted combine in bf16
    (tensor_tensor adds run in the 2x packed mode) with an fp32 final add.
  - Plain coord loads / output stores ride HWDGE (nc.sync/nc.scalar) so the
    GpSimd engine only runs the indirect gathers (it is the bottleneck).
"""

import os
import sys

import numpy as np
import ml_dtypes

for _p in ("/opt/trn_rl_repo",):
    if os.path.isdir(_p) and _p not in sys.path:
        sys.path.insert(0, _p)

from contextlib import ExitStack

import concourse.tile as tile
from concourse import bacc, bass, mybir
from concourse.bass_utils import run_bass_kernel_spmd

F32 = mybir.dt.float32
BF16 = mybir.dt.bfloat16
I32 = mybir.dt.int32

N_POINTS = 2_000_000
C = 32
H = 1024
W = 1024
N_CORES = 8
N_PER_CORE = N_POINTS // N_CORES  # 250_000

P = 128  # partitions
ROW = 2 * C  # pair-table row: 64 bf16 = 128 B
TAB_ROWS = H * W + 64  # pair rows + slack for the (1023,1023)+1 read


def build_program(tile_kqs: tuple, n_pad: int):
    assert n_pad == P * sum(tile_kqs)
    nc = bacc.Bacc(
        "TRN2",
        target_bir_lowering=False,
        debug=False,
        num_devices=N_CORES,
    )
    coord_t = nc.dram_tensor("coord", [n_pad, 2], F32, kind="ExternalInput")
    table_t = nc.dram_tensor("table", [TAB_ROWS, ROW], BF16, kind="ExternalInput")
    out_t = nc.dram_tensor("out", [n_pad, C], F32, kind="ExternalOutput")

    with tile.TileContext(nc) as tc, ExitStack() as ctx:
        coord_pool = ctx.enter_context(tc.tile_pool(name="coord", bufs=3))
        aux_pool = ctx.enter_context(tc.tile_pool(name="aux", bufs=3))
        idx_pool = ctx.enter_context(tc.tile_pool(name="idx", bufs=3))
        w_pool = ctx.enter_context(tc.tile_pool(name="w", bufs=3))
        g_pool = ctx.enter_context(tc.tile_pool(name="g", bufs=2))
        we_pool = ctx.enter_context(tc.tile_pool(name="we", bufs=2))
        a_pool = ctx.enter_context(tc.tile_pool(name="a", bufs=2))
        o_pool = ctx.enter_context(tc.tile_pool(name="o", bufs=2))

        coord_ap = coord_t.ap()
        table_ap = table_t.ap()
        out_ap = out_t.ap()

        q0 = 0
        for it, kq in enumerate(tile_kqs):
            TQ = P * kq

            # ---- load coords [TQ, 2] -> [128, 2*kq]; per-partition layout
            # alternates (x, y) per query. HWDGE (sync) keeps Pool free.
            ct = coord_pool.tile([P, 2 * kq], F32, tag="ct")
            src = coord_ap[q0 : q0 + TQ, :].rearrange("(p k) c -> p (k c)", p=P)
            nc.sync.dma_start(out=ct[:], in_=src)

            # ---- index & weight math (all [128, 2*kq] until the x/y split)
            t = aux_pool.tile([P, 2 * kq], F32, tag="t")
            ti = aux_pool.tile([P, 2 * kq], I32, tag="ti")
            tf = aux_pool.tile([P, 2 * kq], F32, tag="tf")
            t0 = aux_pool.tile([P, 2 * kq], F32, tag="t0")
            fr = aux_pool.tile([P, 2 * kq], F32, tag="fr")
            gr = aux_pool.tile([P, 2 * kq], F32, tag="gr")
            idf = aux_pool.tile([P, kq], F32, tag="idf")

            # t = (coord+1)*0.5*1023 = coord*511.5 + 511.5
            nc.vector.tensor_scalar(
                t[:], ct[:], 511.5, 511.5, mybir.AluOpType.mult, mybir.AluOpType.add
            )
            # floor via int convert + fixup (works for trunc or round-nearest)
            nc.vector.tensor_copy(ti[:], t[:])
            nc.vector.tensor_copy(tf[:], ti[:])
            nc.vector.tensor_tensor(
                out=t0[:], in0=tf[:], in1=t[:], op=mybir.AluOpType.is_gt
            )
            nc.vector.tensor_tensor(
                out=t0[:], in0=tf[:], in1=t0[:], op=mybir.AluOpType.subtract
            )
            nc.vector.tensor_tensor(
                out=fr[:], in0=t[:], in1=t0[:], op=mybir.AluOpType.subtract
            )
            # gr = 1 - fr on the Scalar engine (ACT is otherwise idle)
            nc.scalar.activation(
                gr[:], fr[:], mybir.ActivationFunctionType.Copy, bias=1.0, scale=-1.0
            )

            t3 = t0[:].rearrange("p (k c) -> p k c", c=2)
            x0 = t3[:, :, 0:1].squeeze(2)  # [128, kq] stride-2
            y0 = t3[:, :, 1:2].squeeze(2)
            fr3 = fr[:].rearrange("p (k c) -> p k c", c=2)
            fx = fr3[:, :, 0:1].squeeze(2)
            fy = fr3[:, :, 1:2].squeeze(2)
            gr3 = gr[:].rearrange("p (k c) -> p k c", c=2)
            gx = gr3[:, :, 0:1].squeeze(2)
            gy = gr3[:, :, 1:2].squeeze(2)

            # idx = y0*1024 + x0 (exact in fp32: < 2^20)
            nc.vector.tensor_scalar(
                idf[:], y0, float(W), None, mybir.AluOpType.mult
            )
            nc.vector.tensor_tensor(
                out=idf[:], in0=idf[:], in1=x0, op=mybir.AluOpType.add
            )
            idx = idx_pool.tile([P, kq], I32, tag="idx")
            nc.vector.tensor_copy(idx[:], idf[:])

            # corner weights (bf16), plane layout [128, 4*kq], j = (nw, sw, ne, se)
            wt = w_pool.tile([P, 4 * kq], BF16, tag="wt")
            w_nw = wt[:, 0 * kq : 1 * kq]
            w_sw = wt[:, 1 * kq : 2 * kq]
            w_ne = wt[:, 2 * kq : 3 * kq]
            w_se = wt[:, 3 * kq : 4 * kq]
            nc.vector.tensor_tensor(out=w_nw, in0=gx, in1=gy, op=mybir.AluOpType.mult)
            nc.vector.tensor_tensor(out=w_sw, in0=gx, in1=fy, op=mybir.AluOpType.mult)
            nc.vector.tensor_tensor(out=w_ne, in0=fx, in1=gy, op=mybir.AluOpType.mult)
            nc.vector.tensor_tensor(out=w_se, in0=fx, in1=fy, op=mybir.AluOpType.mult)

            # ---- gather: one index per partition per instruction; each
            # index pulls 128 contiguous bf16 (rows idx, idx+1 = all 4 corners)
            g = g_pool.tile([P, kq * 128], BF16, tag="g")
            for col in range(kq):
                nc.gpsimd.indirect_dma_start(
                    out=g[:, col * 128 : (col + 1) * 128],
                    out_offset=None,
                    in_=table_ap,
                    in_offset=bass.IndirectOffsetOnAxis(
                        ap=idx[:, col : col + 1], axis=0
                    ),
                )

            # ---- combine: g viewed [128, kq, 4, 32], j = (nw, sw, ne, se)
            # Pre-expand the per-corner weights to one-per-element on the
            # Scalar engine (own SBUF ports -> no GpSimd contention), so the
            # big multiply below is stride-1 bf16 on both inputs and runs in
            # the DVE 2x packed mode. A stride-0 broadcast operand would pin
            # it at 1x AND its longer span stalls the Pool engine's SWDGE
            # (TENSOR_TENSOR <-> Q7 SBUF port contention, ~300 us measured).
            wexp = we_pool.tile([P, kq * 128], BF16, tag="wexp")
            w4 = wt[:].rearrange("p (j k) -> p k j", j=4).unsqueeze(3).to_broadcast(
                [P, kq, 4, C]
            )
            wexp4 = wexp[:].rearrange("p (k j c) -> p k j c", j=4, c=C)
            nc.scalar.activation(
                wexp4, w4, mybir.ActivationFunctionType.Copy
            )
            g4 = g[:].rearrange("p (k j c) -> p k j c", j=4, c=C)
            nc.vector.tensor_tensor(
                out=g[:], in0=g[:], in1=wexp[:], op=mybir.AluOpType.mult
            )

            # add1 (bf16, 2x packed): (nw',sw') + (ne',se')
            a = a_pool.tile([P, kq * 64], BF16, tag="a")
            a4 = a[:].rearrange("p (k s c) -> p k s c", s=2, c=C)
            nc.vector.tensor_tensor(
                out=a4, in0=g4[:, :, 0:2, :], in1=g4[:, :, 2:4, :],
                op=mybir.AluOpType.add,
            )
            # add2 -> fp32 out
            o = o_pool.tile([P, kq * C], F32, tag="o")
            o3 = o[:].rearrange("p (k c) -> p k c", c=C)
            nc.vector.tensor_tensor(
                out=o3, in0=a4[:, :, 0:1, :].squeeze(2), in1=a4[:, :, 1:2, :].squeeze(2),
                op=mybir.AluOpType.add,
            )

            # ---- store via HWDGE on the Scalar engine
            dst = out_ap[q0 : q0 + TQ, :].rearrange("(p k) c -> p (k c)", p=P)
            nc.scalar.dma_start(out=dst, in_=o[:])

            q0 += TQ

    nc.compile()
    return nc


# ---------------------------------------------------------------------------
# Host-side wrapper

# 250_000 queries/core need ceil(250_000/128) = 1954 gather instructions;
# 20 tiles of kq=96 plus one of kq=34 hits that exactly with fewer tiles
# (less per-tile fixed overhead: semaphores, DVE/ACT op bubbles).
_TILE_KQS = (96,) * 20 + (34,)
_N_PAD = P * sum(_TILE_KQS)  # 250_112

_nc_cache = {}


def _get_program():
    key = (_TILE_KQS, _N_PAD)
    if key not in _nc_cache:
        _nc_cache[key] = build_program(*key)
    return _nc_cache[key]


def _make_table(params: np.ndarray) -> np.ndarray:
    # G[y, x, c]; pair row (y*W + x) = [G[y, x, :], G[y+1, x, :]] (y clamped)
    g = np.transpose(params[0], (1, 2, 0))  # [H, W, C] fp32
    table = np.zeros((TAB_ROWS, ROW), dtype=ml_dtypes.bfloat16)
    hw = table[: H * W].reshape(H, W, ROW)
    hw[:, :, :C] = g.astype(ml_dtypes.bfloat16)
    hw[:-1, :, C:] = g[1:].astype(ml_dtypes.bfloat16)
    hw[-1, :, C:] = g[-1].astype(ml_dtypes.bfloat16)
    return table


def _run(coord: np.ndarray, params: np.ndarray, trace: bool = False, **kw):
    assert coord.shape == (N_POINTS, 2) and params.shape == (1, C, H, W)
    nc = _get_program()
    table = _make_table(params)

    coord_pad = np.zeros((N_CORES, _N_PAD, 2), dtype=np.float32)
    coord_pad[:, :N_PER_CORE] = coord.reshape(N_CORES, N_PER_CORE, 2)

    in_maps = [
        {"coord": np.ascontiguousarray(coord_pad[c]), "table": table}
        for c in range(N_CORES)
    ]
    res = run_bass_kernel_spmd(nc, in_maps, list(range(N_CORES)), trace=trace, **kw)
    out = np.concatenate(
        [res.results[c]["out"][:N_PER_CORE] for c in range(N_CORES)], axis=0
    )
    return out.astype(np.float32), res


def kernel(coord: np.ndarray, params: np.ndarray) -> np.ndarray:
    return _run(coord, params)[0]


# revision 11
# speedup vs baseline: 1.0265x; 1.0265x over previous
"""Trainium2 Bass kernel: bilinear grid_sample (align_corners=True).

reference: coord [N,2] in [-1,1], params [1,32,1024,1024] -> out [N,32].

Strategy (8 NeuronCores, data-parallel over queries):
  - Host: build a bf16 "pair table" [H*W + pad, 64]: row (y*1024 + x) holds
    the 32 channels of grid pixel (y, x) followed by the 32 channels of
    (y+1, x) (clamped at the bottom edge). Two consecutive rows therefore
    hold all FOUR bilinear corners of a query whose top-left cell is (y, x)
    as 128 contiguous bf16 = 256 B -> ONE indirect-DMA gather per query
    (the baseline needed two), at half the HBM bytes (bf16 vs fp32).
  - Each core: 250k queries, tiled 128 partitions x kq. DVE computes cell
    indices + the 4 corner weights (bf16); one SWDGE indirect DMA per 128
    queries gathers the corner block; DVE does the weighted combine in bf16
    (tensor_tensor adds run in the 2x packed mode) with an fp32 final add.
  - Plain coord loads / output stores ride HWDGE (nc.sync/nc.scalar) so the
    GpSimd engine only runs the indirect gathers (it is the bottleneck).
"""

import os
import sys

import numpy as np
import ml_dtypes

for _p in ("/opt/trn_rl_repo",):
    if os.path.isdir(_p) and _p not in sys.path:
        sys.path.insert(0, _p)

from contextlib import ExitStack

import concourse.tile as tile
from concourse import bacc, bass, mybir
from concourse.bass_utils import run_bass_kernel_spmd

F32 = mybir.dt.float32
BF16 = mybir.dt.bfloat16
I32 = mybir.dt.int32

N_POINTS = 2_000_000
C = 32
H = 1024
W = 1024
N_CORES = 8
N_PER_CORE = N_POINTS // N_CORES  # 250_000

P = 128  # partitions
ROW = 2 * C  # pair-table row: 64 bf16 = 128 B
TAB_ROWS = H * W + 64  # pair rows + slack for the (1023,1023)+1 read


def build_program(tile_kqs: tuple, n_pad: int):
    assert n_pad == P * sum(tile_kqs)
    nc = bacc.Bacc(
        "TRN2",
        target_bir_lowering=False,
        debug=False,
        num_devices=N_CORES,
    )
    coord_t = nc.dram_tensor("coord", [n_pad, 2], F32, kind="ExternalInput")
    table_t = nc.dram_tensor("table", [TAB_ROWS, ROW], BF16, kind="ExternalInput")
    out_t = nc.dram_tensor("out", [n_pad, C], F32, kind="ExternalOutput")

    with tile.TileContext(nc) as tc, ExitStack() as ctx:
        coord_pool = ctx.enter_context(tc.tile_pool(name="coord", bufs=3))
        aux_pool = ctx.enter_context(tc.tile_pool(name="aux", bufs=3))
        idx_pool = ctx.enter_context(tc.tile_pool(name="idx", bufs=3))
        w_pool = ctx.enter_context(tc.tile_pool(name="w", bufs=3))
        g_pool = ctx.enter_context(tc.tile_pool(name="g", bufs=3))
        a_pool = ctx.enter_context(tc.tile_pool(name="a", bufs=2))
        o_pool = ctx.enter_context(tc.tile_pool(name="o", bufs=2))

        coord_ap = coord_t.ap()
        table_ap = table_t.ap()
        out_ap = out_t.ap()

        q0 = 0
        for it, kq in enumerate(tile_kqs):
            TQ = P * kq

            # ---- load coords [TQ, 2] -> [128, 2*kq]; per-partition layout
            # alternates (x, y) per query. HWDGE (sync) keeps Pool free.
            ct = coord_pool.tile([P, 2 * kq], F32, tag="ct")
            src = coord_ap[q0 : q0 + TQ, :].rearrange("(p k) c -> p (k c)", p=P)
            nc.sync.dma_start(out=ct[:], in_=src)

            # ---- index & weight math (all [128, 2*kq] until the x/y split)
            t = aux_pool.tile([P, 2 * kq], F32, tag="t")
            ti = aux_pool.tile([P, 2 * kq], I32, tag="ti")
            tf = aux_pool.tile([P, 2 * kq], F32, tag="tf")
            t0 = aux_pool.tile([P, 2 * kq], F32, tag="t0")
            fr = aux_pool.tile([P, 2 * kq], F32, tag="fr")
            gr = aux_pool.tile([P, 2 * kq], F32, tag="gr")
            idf = aux_pool.tile([P, kq], F32, tag="idf")

            # t = (coord+1)*0.5*1023 = coord*511.5 + 511.5
            nc.vector.tensor_scalar(
                t[:], ct[:], 511.5, 511.5, mybir.AluOpType.mult, mybir.AluOpType.add
            )
            # floor via int convert + fixup (works for trunc or round-nearest)
            nc.vector.tensor_copy(ti[:], t[:])
            nc.vector.tensor_copy(tf[:], ti[:])
            nc.vector.tensor_tensor(
                out=t0[:], in0=tf[:], in1=t[:], op=mybir.AluOpType.is_gt
            )
            nc.vector.tensor_tensor(
                out=t0[:], in0=tf[:], in1=t0[:], op=mybir.AluOpType.subtract
            )
            nc.vector.tensor_tensor(
                out=fr[:], in0=t[:], in1=t0[:], op=mybir.AluOpType.subtract
            )
            # gr = 1 - fr on the Scalar engine (ACT is otherwise idle)
            nc.scalar.activation(
                gr[:], fr[:], mybir.ActivationFunctionType.Copy, bias=1.0, scale=-1.0
            )

            t3 = t0[:].rearrange("p (k c) -> p k c", c=2)
            x0 = t3[:, :, 0:1].squeeze(2)  # [128, kq] stride-2
            y0 = t3[:, :, 1:2].squeeze(2)
            fr3 = fr[:].rearrange("p (k c) -> p k c", c=2)
            fx = fr3[:, :, 0:1].squeeze(2)
            fy = fr3[:, :, 1:2].squeeze(2)
            gr3 = gr[:].rearrange("p (k c) -> p k c", c=2)
            gx = gr3[:, :, 0:1].squeeze(2)
            gy = gr3[:, :, 1:2].squeeze(2)

            # idx = y0*1024 + x0 (exact in fp32: < 2^20)
            nc.vector.tensor_scalar(
                idf[:], y0, float(W), None, mybir.AluOpType.mult
            )
            nc.vector.tensor_tensor(
                out=idf[:], in0=idf[:], in1=x0, op=mybir.AluOpType.add
            )
            idx = idx_pool.tile([P, kq], I32, tag="idx")
            nc.vector.tensor_copy(idx[:], idf[:])

            # corner weights (bf16), plane layout [128, 4*kq], j = (nw, sw, ne, se)
            wt = w_pool.tile([P, 4 * kq], BF16, tag="wt")
            w_nw = wt[:, 0 * kq : 1 * kq]
            w_sw = wt[:, 1 * kq : 2 * kq]
            w_ne = wt[:, 2 * kq : 3 * kq]
            w_se = wt[:, 3 * kq : 4 * kq]
            nc.vector.tensor_tensor(out=w_nw, in0=gx, in1=gy, op=mybir.AluOpType.mult)
            nc.vector.tensor_tensor(out=w_sw, in0=gx, in1=fy, op=mybir.AluOpType.mult)
            nc.vector.tensor_tensor(out=w_ne, in0=fx, in1=gy, op=mybir.AluOpType.mult)
            nc.vector.tensor_tensor(out=w_se, in0=fx, in1=fy, op=mybir.AluOpType.mult)

            # ---- gather: one index per partition per instruction; each
            # index pulls 128 contiguous bf16 (rows idx, idx+1 = all 4 corners)
            g = g_pool.tile([P, kq * 128], BF16, tag="g")
            for col in range(kq):
                nc.gpsimd.indirect_dma_start(
                    out=g[:, col * 128 : (col + 1) * 128],
                    out_offset=None,
                    in_=table_ap,
                    in_offset=bass.IndirectOffsetOnAxis(
                        ap=idx[:, col : col + 1], axis=0
                    ),
                )

            # ---- combine: g viewed [128, kq, 4, 32], j = (nw, sw, ne, se)
            # Pre-expand the per-corner weights to one-per-element on the
            # Scalar engine (own SBUF ports -> no GpSimd contention), so the
            # big multiply below is stride-1 bf16 on both inputs and runs in
            # the DVE 2x packed mode. A stride-0 broadcast operand would pin
            # it at 1x AND its longer span stalls the Pool engine's SWDGE
            # (TENSOR_TENSOR <-> Q7 SBUF port contention, ~300 us measured).
            wexp = w_pool.tile([P, kq * 128], BF16, tag="wexp")
            w4 = wt[:].rearrange("p (j k) -> p k j", j=4).unsqueeze(3).to_broadcast(
                [P, kq, 4, C]
            )
            # All long spans are chopped into ~1 us chunks: a gather's Q7
            # SBUF phase only stalls badly when a long op covers it entirely
            # (measured: short CAST/TS ops cause ~zero inflation, long
            # TT/ACTIVATE spans inflate gathers at ~0.37/0.24 us per us).
            wexp4 = wexp[:].rearrange("p (k j c) -> p k j c", j=4, c=C)
            for k0 in range(0, kq, 8):
                k1 = min(k0 + 8, kq)
                nc.scalar.activation(
                    wexp4[:, k0:k1, :, :], w4[:, k0:k1, :, :],
                    mybir.ActivationFunctionType.Copy,
                )
            g4 = g[:].rearrange("p (k j c) -> p k j c", j=4, c=C)
            for k0 in range(0, kq, 16):
                k1 = min(k0 + 16, kq)
                nc.vector.tensor_tensor(
                    out=g[:, k0 * 128 : k1 * 128],
                    in0=g[:, k0 * 128 : k1 * 128],
                    in1=wexp[:, k0 * 128 : k1 * 128],
                    op=mybir.AluOpType.mult,
                )

            # add1 (bf16, 2x packed): (nw',sw') + (ne',se')
            a = a_pool.tile([P, kq * 64], BF16, tag="a")
            a4 = a[:].rearrange("p (k s c) -> p k s c", s=2, c=C)
            for k0 in range(0, kq, 32):
                k1 = min(k0 + 32, kq)
                nc.vector.tensor_tensor(
                    out=a4[:, k0:k1, :, :], in0=g4[:, k0:k1, 0:2, :],
                    in1=g4[:, k0:k1, 2:4, :], op=mybir.AluOpType.add,
                )
            # add2 -> fp32 out
            o = o_pool.tile([P, kq * C], F32, tag="o")
            o3 = o[:].rearrange("p (k c) -> p k c", c=C)
            for k0 in range(0, kq, 32):
                k1 = min(k0 + 32, kq)
                nc.vector.tensor_tensor(
                    out=o3[:, k0:k1, :],
                    in0=a4[:, k0:k1, 0:1, :].squeeze(2),
                    in1=a4[:, k0:k1, 1:2, :].squeeze(2),
                    op=mybir.AluOpType.add,
                )

            # ---- store via HWDGE on the Scalar engine
            dst = out_ap[q0 : q0 + TQ, :].rearrange("(p k) c -> p (k c)", p=P)
            nc.scalar.dma_start(out=dst, in_=o[:])

            q0 += TQ

    nc.compile()
    return nc


# ---------------------------------------------------------------------------
# Host-side wrapper

# 250_000 queries/core need ceil(250_000/128) = 1954 gather instructions;
# 30 tiles of kq=64 plus one of kq=34 hits that exactly (vs 31x64 = 1984).
_TILE_KQS = (64,) * 30 + (34,)
_N_PAD = P * sum(_TILE_KQS)  # 250_112

_nc_cache = {}


def _get_program():
    key = (_TILE_KQS, _N_PAD)
    if key not in _nc_cache:
        _nc_cache[key] = build_program(*key)
    return _nc_cache[key]


def _make_table(params: np.ndarray) -> np.ndarray:
    # G[y, x, c]; pair row (y*W + x) = [G[y, x, :], G[y+1, x, :]] (y clamped)
    g = np.transpose(params[0], (1, 2, 0))  # [H, W, C] fp32
    table = np.zeros((TAB_ROWS, ROW), dtype=ml_dtypes.bfloat16)
    hw = table[: H * W].reshape(H, W, ROW)
    hw[:, :, :C] = g.astype(ml_dtypes.bfloat16)
    hw[:-1, :, C:] = g[1:].astype(ml_dtypes.bfloat16)
    hw[-1, :, C:] = g[-1].astype(ml_dtypes.bfloat16)
    return table


def _run(coord: np.ndarray, params: np.ndarray, trace: bool = False, **kw):
    assert coord.shape == (N_POINTS, 2) and params.shape == (1, C, H, W)
    nc = _get_program()
    table = _make_table(params)

    coord_pad = np.zeros((N_CORES, _N_PAD, 2), dtype=np.float32)
    coord_pad[:, :N_PER_CORE] = coord.reshape(N_CORES, N_PER_CORE, 2)

    in_maps = [
        {"coord": np.ascontiguousarray(coord_pad[c]), "table": table}
        for c in range(N_CORES)
    ]
    res = run_bass_kernel_spmd(nc, in_maps, list(range(N_CORES)), trace=trace, **kw)
    out = np.concatenate(
        [res.results[c]["out"][:N_PER_CORE] for c in range(N_CORES)], axis=0
    )
    return out.astype(np.float32), res


def kernel(coord: np.ndarray, params: np.ndarray) -> np.ndarray:
    return _run(coord, params)[0]


# revision 12
# speedup vs baseline: 1.0337x; 1.0070x over previous
"""Trainium2 Bass kernel: bilinear grid_sample (align_corners=True).

reference: coord [N,2] in [-1,1], params [1,32,1024,1024] -> out [N,32].

Strategy (8 NeuronCores, data-parallel over queries):
  - Host: build a bf16 "pair table" [H*W + pad, 64]: row (y*1024 + x) holds
    the 32 channels of grid pixel (y, x) followed by the 32 channels of
    (y+1, x) (clamped at the bottom edge). Two consecutive rows therefore
    hold all FOUR bilinear corners of a query whose top-left cell is (y, x)
    as 128 contiguous bf16 = 256 B -> ONE indirect-DMA gather per query
    (the baseline needed two), at half the HBM bytes (bf16 vs fp32).
  - Each core: 250k queries, tiled 128 partitions x kq. DVE computes cell
    indices + the 4 corner weights (bf16); one SWDGE indirect DMA per 128
    queries gathers the corner block; DVE does the weighted combine in bf16
    (tensor_tensor adds run in the 2x packed mode) with an fp32 final add.
  - Plain coord loads / output stores ride HWDGE (nc.sync/nc.scalar) so the
    GpSimd engine only runs the indirect gathers (it is the bottleneck).
"""

import os
import sys

import numpy as np
import ml_dtypes

for _p in ("/opt/trn_rl_repo",):
    if os.path.isdir(_p) and _p not in sys.path:
        sys.path.insert(0, _p)

from contextlib import ExitStack

import concourse.tile as tile
from concourse import bacc, bass, mybir
from concourse.bass_utils import run_bass_kernel_spmd

F32 = mybir.dt.float32
BF16 = mybir.dt.bfloat16
I32 = mybir.dt.int32

N_POINTS = 2_000_000
C = 32
H = 1024
W = 1024
N_CORES = 8
N_PER_CORE = N_POINTS // N_CORES  # 250_000

P = 128  # partitions
ROW = 2 * C  # pair-table row: 64 bf16 = 128 B
TAB_ROWS = H * W + 64  # pair rows + slack for the (1023,1023)+1 read


def build_program(tile_kqs: tuple, n_pad: int):
    assert n_pad == P * sum(tile_kqs)
    nc = bacc.Bacc(
        "TRN2",
        target_bir_lowering=False,
        debug=False,
        num_devices=N_CORES,
    )
    coord_t = nc.dram_tensor("coord", [n_pad, 2], F32, kind="ExternalInput")
    table_t = nc.dram_tensor("table", [TAB_ROWS, ROW], BF16, kind="ExternalInput")
    out_t = nc.dram_tensor("out", [n_pad, C], F32, kind="ExternalOutput")

    with tile.TileContext(nc) as tc, ExitStack() as ctx:
        coord_pool = ctx.enter_context(tc.tile_pool(name="coord", bufs=3))
        aux_pool = ctx.enter_context(tc.tile_pool(name="aux", bufs=3))
        idx_pool = ctx.enter_context(tc.tile_pool(name="idx", bufs=3))
        w_pool = ctx.enter_context(tc.tile_pool(name="w", bufs=3))
        g_pool = ctx.enter_context(tc.tile_pool(name="g", bufs=3))
        a_pool = ctx.enter_context(tc.tile_pool(name="a", bufs=2))
        o_pool = ctx.enter_context(tc.tile_pool(name="o", bufs=2))

        coord_ap = coord_t.ap()
        table_ap = table_t.ap()
        out_ap = out_t.ap()

        q0 = 0
        for it, kq in enumerate(tile_kqs):
            TQ = P * kq

            # ---- load coords [TQ, 2] -> [128, 2*kq]; per-partition layout
            # alternates (x, y) per query. HWDGE (sync) keeps Pool free.
            ct = coord_pool.tile([P, 2 * kq], F32, tag="ct")
            src = coord_ap[q0 : q0 + TQ, :].rearrange("(p k) c -> p (k c)", p=P)
            nc.sync.dma_start(out=ct[:], in_=src)

            # ---- index & weight math (all [128, 2*kq] until the x/y split)
            t = aux_pool.tile([P, 2 * kq], F32, tag="t")
            ti = aux_pool.tile([P, 2 * kq], I32, tag="ti")
            tf = aux_pool.tile([P, 2 * kq], F32, tag="tf")
            t0 = aux_pool.tile([P, 2 * kq], F32, tag="t0")
            fr = aux_pool.tile([P, 2 * kq], F32, tag="fr")
            gr = aux_pool.tile([P, 2 * kq], F32, tag="gr")
            idf = aux_pool.tile([P, kq], F32, tag="idf")

            # t = (coord+1)*0.5*1023 = coord*511.5 + 511.5
            nc.vector.tensor_scalar(
                t[:], ct[:], 511.5, 511.5, mybir.AluOpType.mult, mybir.AluOpType.add
            )
            # floor via int convert + fixup (works for trunc or round-nearest)
            nc.vector.tensor_copy(ti[:], t[:])
            nc.vector.tensor_copy(tf[:], ti[:])
            nc.vector.tensor_tensor(
                out=t0[:], in0=tf[:], in1=t[:], op=mybir.AluOpType.is_gt
            )
            nc.vector.tensor_tensor(
                out=t0[:], in0=tf[:], in1=t0[:], op=mybir.AluOpType.subtract
            )
            nc.vector.tensor_tensor(
                out=fr[:], in0=t[:], in1=t0[:], op=mybir.AluOpType.subtract
            )
            # gr = 1 - fr on the Scalar engine (ACT is otherwise idle)
            nc.scalar.activation(
                gr[:], fr[:], mybir.ActivationFunctionType.Copy, bias=1.0, scale=-1.0
            )

            t3 = t0[:].rearrange("p (k c) -> p k c", c=2)
            x0 = t3[:, :, 0:1].squeeze(2)  # [128, kq] stride-2
            y0 = t3[:, :, 1:2].squeeze(2)
            fr3 = fr[:].rearrange("p (k c) -> p k c", c=2)
            fx = fr3[:, :, 0:1].squeeze(2)
            fy = fr3[:, :, 1:2].squeeze(2)
            gr3 = gr[:].rearrange("p (k c) -> p k c", c=2)
            gx = gr3[:, :, 0:1].squeeze(2)
            gy = gr3[:, :, 1:2].squeeze(2)

            # idx = y0*1024 + x0 (exact in fp32: < 2^20)
            nc.vector.tensor_scalar(
                idf[:], y0, float(W), None, mybir.AluOpType.mult
            )
            nc.vector.tensor_tensor(
                out=idf[:], in0=idf[:], in1=x0, op=mybir.AluOpType.add
            )
            idx = idx_pool.tile([P, kq], I32, tag="idx")
            nc.vector.tensor_copy(idx[:], idf[:])

            # corner weights (bf16), plane layout [128, 4*kq], j = (nw, sw, ne, se)
            wt = w_pool.tile([P, 4 * kq], BF16, tag="wt")
            w_nw = wt[:, 0 * kq : 1 * kq]
            w_sw = wt[:, 1 * kq : 2 * kq]
            w_ne = wt[:, 2 * kq : 3 * kq]
            w_se = wt[:, 3 * kq : 4 * kq]
            nc.vector.tensor_tensor(out=w_nw, in0=gx, in1=gy, op=mybir.AluOpType.mult)
            nc.vector.tensor_tensor(out=w_sw, in0=gx, in1=fy, op=mybir.AluOpType.mult)
            nc.vector.tensor_tensor(out=w_ne, in0=fx, in1=gy, op=mybir.AluOpType.mult)
            nc.vector.tensor_tensor(out=w_se, in0=fx, in1=fy, op=mybir.AluOpType.mult)

            # ---- gather: one index per partition per instruction; each
            # index pulls 128 contiguous bf16 (rows idx, idx+1 = all 4 corners)
            g = g_pool.tile([P, kq * 128], BF16, tag="g")
            for col in range(kq):
                nc.gpsimd.indirect_dma_start(
                    out=g[:, col * 128 : (col + 1) * 128],
                    out_offset=None,
                    in_=table_ap,
                    in_offset=bass.IndirectOffsetOnAxis(
                        ap=idx[:, col : col + 1], axis=0
                    ),
                )

            # ---- combine: g viewed [128, kq, 4, 32], j = (nw, sw, ne, se)
            # Pre-expand the per-corner weights to one-per-element on the
            # Scalar engine (own SBUF ports -> no GpSimd contention), so the
            # big multiply below is stride-1 bf16 on both inputs and runs in
            # the DVE 2x packed mode. A stride-0 broadcast operand would pin
            # it at 1x AND its longer span stalls the Pool engine's SWDGE
            # (TENSOR_TENSOR <-> Q7 SBUF port contention, ~300 us measured).
            wexp = w_pool.tile([P, kq * 128], BF16, tag="wexp")
            w4 = wt[:].rearrange("p (j k) -> p k j", j=4).unsqueeze(3).to_broadcast(
                [P, kq, 4, C]
            )
            # All long spans are chopped into ~1 us chunks: a gather's Q7
            # SBUF phase only stalls badly when a long op covers it entirely
            # (measured: short CAST/TS ops cause ~zero inflation, long
            # TT/ACTIVATE spans inflate gathers at ~0.37/0.24 us per us).
            wexp4 = wexp[:].rearrange("p (k j c) -> p k j c", j=4, c=C)
            for k0 in range(0, kq, 4):
                k1 = min(k0 + 4, kq)
                nc.scalar.activation(
                    wexp4[:, k0:k1, :, :], w4[:, k0:k1, :, :],
                    mybir.ActivationFunctionType.Copy,
                )
            g4 = g[:].rearrange("p (k j c) -> p k j c", j=4, c=C)
            for k0 in range(0, kq, 8):
                k1 = min(k0 + 8, kq)
                nc.vector.tensor_tensor(
                    out=g[:, k0 * 128 : k1 * 128],
                    in0=g[:, k0 * 128 : k1 * 128],
                    in1=wexp[:, k0 * 128 : k1 * 128],
                    op=mybir.AluOpType.mult,
                )

            # add1 (bf16, 2x packed): (nw',sw') + (ne',se')
            a = a_pool.tile([P, kq * 64], BF16, tag="a")
            a4 = a[:].rearrange("p (k s c) -> p k s c", s=2, c=C)
            for k0 in range(0, kq, 16):
                k1 = min(k0 + 16, kq)
                nc.vector.tensor_tensor(
                    out=a4[:, k0:k1, :, :], in0=g4[:, k0:k1, 0:2, :],
                    in1=g4[:, k0:k1, 2:4, :], op=mybir.AluOpType.add,
                )
            # add2 -> fp32 out
            o = o_pool.tile([P, kq * C], F32, tag="o")
            o3 = o[:].rearrange("p (k c) -> p k c", c=C)
            for k0 in range(0, kq, 16):
                k1 = min(k0 + 16, kq)
                nc.vector.tensor_tensor(
                    out=o3[:, k0:k1, :],
                    in0=a4[:, k0:k1, 0:1, :].squeeze(2),
                    in1=a4[:, k0:k1, 1:2, :].squeeze(2),
                    op=mybir.AluOpType.add,
                )

            # ---- store via HWDGE on the Scalar engine
            dst = out_ap[q0 : q0 + TQ, :].rearrange("(p k) c -> p (k c)", p=P)
            nc.scalar.dma_start(out=dst, in_=o[:])

            q0 += TQ

    nc.compile()
    return nc


# ---------------------------------------------------------------------------
# Host-side wrapper

# 250_000 queries/core need ceil(250_000/128) = 1954 gather instructions;
# 30 tiles of kq=64 plus one of kq=34 hits that exactly (vs 31x64 = 1984).
_TILE_KQS = (64,) * 30 + (34,)
_N_PAD = P * sum(_TILE_KQS)  # 250_112

_nc_cache = {}


def _get_program():
    key = (_TILE_KQS, _N_PAD)
    if key not in _nc_cache:
        _nc_cache[key] = build_program(*key)
    return _nc_cache[key]


def _make_table(params: np.ndarray) -> np.ndarray:
    # G[y, x, c]; pair row (y*W + x) = [G[y, x, :], G[y+1, x, :]] (y clamped)
    g = np.transpose(params[0], (1, 2, 0))  # [H, W, C] fp32
    table = np.zeros((TAB_ROWS, ROW), dtype=ml_dtypes.bfloat16)
    hw = table[: H * W].reshape(H, W, ROW)
    hw[:, :, :C] = g.astype(ml_dtypes.bfloat16)
    hw[:-1, :, C:] = g[1:].astype(ml_dtypes.bfloat16)
    hw[-1, :, C:] = g[-1].astype(ml_dtypes.bfloat16)
    return table


def _run(coord: np.ndarray, params: np.ndarray, trace: bool = False, **kw):
    assert coord.shape == (N_POINTS, 2) and params.shape == (1, C, H, W)
    nc = _get_program()
    table = _make_table(params)

    coord_pad = np.zeros((N_CORES, _N_PAD, 2), dtype=np.float32)
    coord_pad[:, :N_PER_CORE] = coord.reshape(N_CORES, N_PER_CORE, 2)

    in_maps = [
        {"coord": np.ascontiguousarray(coord_pad[c]), "table": table}
        for c in range(N_CORES)
    ]
    res = run_bass_kernel_spmd(nc, in_maps, list(range(N_CORES)), trace=trace, **kw)
    out = np.concatenate(
        [res.results[c]["out"][:N_PER_CORE] for c in range(N_CORES)], axis=0
    )
    return out.astype(np.float32), res


def kernel(coord: np.ndarray, params: np.ndarray) -> np.ndarray:
    return _run(coord, params)[0]
